# revision 12
# baseline (speedup 1.0000x reference)
"""Contrastive (InfoNCE) loss kernel for Trainium2, 8 NeuronCores.

Strategy (data-parallel over z1 rows, per the sharding hint):
  - Core k owns rows [k*1024, (k+1)*1024) of view1 and receives ALL of z2
    (view2's normalized form) -- one shared fp8 array, no per-core copy.
    z2 is normalized once on the host (the stand-in for "all-gather z2 or
    its normalized form"), scaled by 16, quantized to fp8e4m3, and
    pre-arranged into SBUF tile order so every DMA moves contiguous runs.
  - view1's slab ships RAW fp8 (quantize-then-normalize: the row norm of
    the quantized slab folds into the exp as the per-partition scale
    a1_i = 1/(8*||x1q_i||), so x1 needs NO on-device prescale pass).
    a1 and the similarity diagonal (both O(N*D) scalars of the exact fp8
    operands) are computed host-side and shipped as two [P, IT] tensors.
  - Per core the NEFF is a pure compute stream over column blocks
    (widths 512..2048 -- narrow at the edges to shorten the DMA ramp-in
    and the exp/reduce tail):
      sim tile [128, W] = x1_tile.T @ z2_tile  (fp8 DoubleRow matmuls,
        two 128-deep k-tiles per instruction, fp32 PSUM accum;
        sim = 16*||x1q_i||*cos)
      exp in one ACT op with per-partition scale AP a1[:, it]
        (no max subtraction: |logits| <= ~2.2); row-sum on DVE.
      row_loss = ln(sum_j exp) - s_diag
  - Host sums the 8192 per-row losses and divides by N.
"""

import numpy as np

import concourse.bass as bass
import concourse.mybir as mybir
import concourse.tile as tile
from concourse import bacc
from concourse.bass_utils import run_bass_kernel_spmd
from concourse.hw_specs import get_activation_tables


class _BaccOneActSet(bacc.Bacc):
    """Bacc whose act-table pass may only pick natural_log_exp_and_others.

    The default greedy picker ping-pongs between table sets, costing a
    ~1.3us table load per switch. Both functions used here (Exp, Ln) live
    in natural_log_exp_and_others, so masking the other sets (indices
    preserved) yields a single hoisted load.
    """

    ACT_SET = "natural_log_exp_and_others"

    def insert_act_table_loads(self):
        has_activation = any(
            isinstance(i, mybir.InstActivation)
            for b in self.main_func.blocks
            for i in b.instructions
        )
        if not has_activation:
            return
        tables = [
            (n, (s if n == self.ACT_SET else set()))
            for n, s in get_activation_tables(self.m.arch).items()
        ]
        bacc._bass_rust.insert_act_table_loads(self, tables)

N, D = 8192, 1024
NC = 8
NLOC = N // NC            # rows of view1 per core
P = 128                   # SBUF partitions
KT = D // P               # contraction tiles (128-deep)
KTP = KT // 2             # DoubleRow pairs of contraction tiles
IT = NLOC // P            # output row tiles per core
MMW = 512                 # PSUM free width per DoubleRow matmul
# similarity-column block widths; sum = N. Narrow head (start computing
# after a small DMA) and narrow tail (short exp/reduce epilogue).
BLKS = [512, 1536, 2048, 2048, 1536, 512]
assert sum(BLKS) == N
NBLK = len(BLKS)
GAM = 16.0                # fp8 headroom scale on z2
# sim PSUM value G = x1q . (GAM * z2hat) = GAM * ||x1q_i|| * cos
# logit = 2*cos = G / (8 * ||x1q_i||)  ->  exp scale a1_i = 1/(8*||x1q_i||)

F32 = mybir.dt.float32
BF16 = mybir.dt.bfloat16
FP8 = mybir.dt.float8e4
AF = mybir.ActivationFunctionType
DR = mybir.MatmulPerfMode.DoubleRow


def build_bass(reps: int = 1):
    # reps>1 repeats the (idempotent) compute for device-time slope timing
    nc = _BaccOneActSet("TRN2", target_bir_lowering=False, debug=False)
    # All inputs pre-arranged on host into SBUF tile order: partition-major,
    # contiguous free dim -> DMA moves large contiguous runs per partition.
    # x1 is it-major so the first sim tile only needs a 128KB chunk.
    x1t = nc.dram_tensor("x1t", [P, IT * KT * P], FP8, kind="ExternalInput")
    z2t = nc.dram_tensor("z2t", [P, KT * N], FP8, kind="ExternalInput")
    a1t = nc.dram_tensor("a1t", [P, IT], F32, kind="ExternalInput")
    sdt = nc.dram_tensor("sdt", [P, IT], F32, kind="ExternalInput")
    out = nc.dram_tensor("row_loss", [P, IT], F32, kind="ExternalOutput")

    with tile.TileContext(nc) as tc:
        with (
            tc.tile_pool(name="x1", bufs=1) as x1pool,
            tc.tile_pool(name="z2", bufs=3) as z2pool,
            tc.tile_pool(name="dump", bufs=3) as dumppool,
            tc.tile_pool(name="small", bufs=1) as small,
            tc.tile_pool(name="psim", bufs=2, space="PSUM") as psim,
        ):
            # x1 + per-row scalars issue from the Scalar DGE queue so their
            # descriptor generation overlaps the Sync queue's z2 block 0.
            x1s = x1pool.tile([P, IT, KT, P], FP8)
            x1r = x1t.ap().rearrange("p (h f) -> p h f", h=2)
            nc.scalar.dma_start(
                out=x1s[:, 0:IT // 2].rearrange("p a b c -> p (a b c)"),
                in_=x1r[:, 0, :],
            )
            nc.scalar.dma_start(
                out=x1s[:, IT // 2:].rearrange("p a b c -> p (a b c)"),
                in_=x1r[:, 1, :],
            )
            a1 = small.tile([P, IT], F32)
            nc.scalar.dma_start(out=a1, in_=a1t.ap())
            sdiag = small.tile([P, IT], F32)
            nc.scalar.dma_start(out=sdiag, in_=sdt.ap())

            expsums = small.tile([P, IT, NBLK], F32)

            # ---- stream z2 by column blocks
            offs = np.cumsum([0] + BLKS)[:-1]
            sched = [(int(o), w) for o, w in zip(offs, BLKS)] * reps
            for jb, (off, w) in enumerate(sched):
                z2s = z2pool.tile([P, KT, w], FP8, tag=f"z2w{w}")
                nc.sync.dma_start(
                    out=z2s,
                    in_=z2t.ap()[:, KT * off:KT * (off + w)].rearrange(
                        "p (kt w) -> p kt w", kt=KT
                    ),
                )

                # ---- similarity block + exp + row-sum (fp8 DoubleRow)
                for it in range(IT):
                    sim = psim.tile([P, w], F32, tag="sim")
                    for ktp in range(KTP):
                        for h in range(w // MMW):
                            nc.tensor.matmul(
                                sim[:, h * MMW:(h + 1) * MMW],
                                x1s[:, it, 2 * ktp:2 * ktp + 2, :],
                                z2s[:, 2 * ktp:2 * ktp + 2,
                                    h * MMW:(h + 1) * MMW],
                                start=(ktp == 0),
                                stop=(ktp == KTP - 1),
                                perf_mode=DR,
                            )
                    dump = dumppool.tile([P, w], BF16, tag=f"dw{w}")
                    nc.scalar.activation(
                        dump, sim, AF.Exp, scale=a1[:, it:it + 1],
                    )
                    nc.vector.reduce_sum(
                        expsums[:, it, jb % NBLK:jb % NBLK + 1], dump,
                        axis=mybir.AxisListType.X,
                    )

            # ---- epilogue: row_loss = ln(sum_j exp) - s_diag
            s = small.tile([P, IT], F32)
            nc.vector.reduce_sum(s, expsums, axis=mybir.AxisListType.X)
            lse = small.tile([P, IT], F32)
            nc.scalar.activation(lse, s, AF.Ln)
            rl = small.tile([P, IT], F32)
            nc.vector.tensor_sub(rl, lse, sdiag)
            nc.sync.dma_start(out=out.ap(), in_=rl)

    nc.compile()
    return nc


_NC_CACHE = None
_LAST_RESULTS = None
_NORM_JIT = None


def _host_prep(view1: np.ndarray, view2: np.ndarray):
    """Normalize z2 once on host (the all-gather stand-in), quantize both
    operands to fp8, pre-arrange into SBUF tile order, and compute the
    per-row exp scales + similarity diagonal of the exact fp8 values."""
    global _NORM_JIT
    import jax
    import ml_dtypes

    fp8 = np.dtype(ml_dtypes.float8_e4m3)
    cpu = jax.devices("cpu")[0]
    if _NORM_JIT is None:
        import jax.numpy as jnp

        def _norm_t(v2):
            # [N, D] -> normalized, scaled, transposed [D, N]
            n = jnp.sqrt(jnp.sum(v2 * v2, axis=1, keepdims=True))
            z = v2 * (GAM / jnp.maximum(n, 1e-12))
            return z.T

        _NORM_JIT = jax.jit(_norm_t, backend="cpu")
    with jax.default_device(cpu):
        z2T = np.asarray(_NORM_JIT(view2))       # [D, N] f32
    z2T8 = z2T.astype(fp8)
    x1T8 = np.ascontiguousarray(
        np.asarray(view1, np.float32).T
    ).astype(fp8)                                # [D, N]

    x1f = x1T8.astype(np.float32)                # exact fp8 values
    z2f = z2T8.astype(np.float32)
    nsq1 = np.einsum("di,di->i", x1f, x1f)       # ||x1q_i||^2
    a1 = 1.0 / (8.0 * np.sqrt(nsq1))             # exp scale per row
    sdiag = a1 * np.einsum("di,di->i", x1f, z2f)  # true logit diagonal

    # z2: [D, N] -> block-major [P, sum_b KT*w_b]: each column block is one
    # contiguous kt-major run per partition (up to 16KB descriptors)
    offs = np.cumsum([0] + BLKS)[:-1]
    z2full = np.concatenate(
        [
            np.ascontiguousarray(
                z2T8[:, o:o + w].reshape(KT, P, w).transpose(1, 0, 2)
            ).reshape(P, KT * w)
            for o, w in zip(offs, BLKS)
        ],
        axis=1,
    )

    def x1_tiles(a):  # [D, NLOC] -> [P, IT, KT, 128] it-major
        return np.ascontiguousarray(
            a.reshape(KT, P, IT, P).transpose(1, 2, 0, 3)
        ).reshape(P, -1)

    def pcol(v):  # [NLOC] -> [P, IT] with row it*128+p at [p, it]
        return np.ascontiguousarray(v.reshape(IT, P).T.astype(np.float32))

    return x1T8, z2full, a1, sdiag, x1_tiles, pcol


def kernel(view1: np.ndarray, view2: np.ndarray) -> np.ndarray:
    global _NC_CACHE
    x1 = np.asarray(view1, dtype=np.float32)
    x2 = np.asarray(view2, dtype=np.float32)
    assert x1.shape == (N, D) and x2.shape == (N, D)

    x1T8, z2full, a1, sdiag, x1_tiles, pcol = _host_prep(x1, x2)

    in_maps = []
    for k in range(NC):
        sl = slice(k * NLOC, (k + 1) * NLOC)
        in_maps.append({
            "x1t": x1_tiles(x1T8[:, sl]),
            "z2t": z2full,
            "a1t": pcol(a1[sl]),
            "sdt": pcol(sdiag[sl]),
        })

    if _NC_CACHE is None:
        _NC_CACHE = build_bass()
    res = run_bass_kernel_spmd(_NC_CACHE, in_maps, core_ids=list(range(NC)))
    global _LAST_RESULTS
    _LAST_RESULTS = res

    total = 0.0
    for k in range(NC):
        total += res.results[k]["row_loss"].astype(np.float64).sum()
    return np.float32(total / N)


# revision 13
# speedup vs baseline: 1.0100x; 1.0100x over previous
"""Contrastive (InfoNCE) loss kernel for Trainium2, 8 NeuronCores.

Strategy (data-parallel over z1 rows, per the sharding hint):
  - Core k owns rows [k*1024, (k+1)*1024) of view1 and receives ALL of z2
    (view2's normalized form) -- one shared fp8 array, no per-core copy.
    z2 is normalized once on the host (the stand-in for "all-gather z2 or
    its normalized form"), scaled by 16, quantized to fp8e4m3, and
    pre-arranged into SBUF tile order so every DMA moves contiguous runs.
  - view1's slab ships RAW fp8 (quantize-then-normalize: the row norm of
    the quantized slab folds into the exp as the per-partition scale
    a1_i = 1/(8*||x1q_i||), so x1 needs NO on-device prescale pass).
    a1 and the similarity diagonal (both O(N*D) scalars of the exact fp8
    operands) are computed host-side and shipped as two [P, IT] tensors.
  - Per core the NEFF is a pure compute stream over column blocks
    (widths 512..2048 -- narrow at the edges to shorten the DMA ramp-in
    and the exp/reduce tail):
      sim tile [128, W] = x1_tile.T @ z2_tile  (fp8 DoubleRow matmuls,
        two 128-deep k-tiles per instruction, fp32 PSUM accum;
        sim = 16*||x1q_i||*cos)
      exp in one ACT op with per-partition scale AP a1[:, it]
        (no max subtraction: |logits| <= ~2.2); row-sum on DVE.
      row_loss = ln(sum_j exp) - s_diag
  - Host sums the 8192 per-row losses and divides by N.
"""

import numpy as np

import concourse.bass as bass
import concourse.mybir as mybir
import concourse.tile as tile
from concourse import bacc
from concourse.bass_utils import run_bass_kernel_spmd
from concourse.hw_specs import get_activation_tables


class _BaccOneActSet(bacc.Bacc):
    """Bacc whose act-table pass may only pick natural_log_exp_and_others.

    The default greedy picker ping-pongs between table sets, costing a
    ~1.3us table load per switch. Both functions used here (Exp, Ln) live
    in natural_log_exp_and_others, so masking the other sets (indices
    preserved) yields a single hoisted load.
    """

    ACT_SET = "natural_log_exp_and_others"

    def insert_act_table_loads(self):
        has_activation = any(
            isinstance(i, mybir.InstActivation)
            for b in self.main_func.blocks
            for i in b.instructions
        )
        if not has_activation:
            return
        tables = [
            (n, (s if n == self.ACT_SET else set()))
            for n, s in get_activation_tables(self.m.arch).items()
        ]
        bacc._bass_rust.insert_act_table_loads(self, tables)

N, D = 8192, 1024
NC = 8
NLOC = N // NC            # rows of view1 per core
P = 128                   # SBUF partitions
KT = D // P               # contraction tiles (128-deep)
KTP = KT // 2             # DoubleRow pairs of contraction tiles
IT = NLOC // P            # output row tiles per core
MMW = 512                 # PSUM free width per DoubleRow matmul
# similarity-column block widths; sum = N. Narrow head (start computing
# after a small DMA) and narrow tail (short exp/reduce epilogue).
BLKS = [512, 1536, 2048, 2048, 1536, 512]
assert sum(BLKS) == N
NBLK = len(BLKS)
GAM = 16.0                # fp8 headroom scale on z2
# sim PSUM value G = x1q . (GAM * z2hat) = GAM * ||x1q_i|| * cos
# logit = 2*cos = G / (8 * ||x1q_i||)  ->  exp scale a1_i = 1/(8*||x1q_i||)

F32 = mybir.dt.float32
BF16 = mybir.dt.bfloat16
FP8 = mybir.dt.float8e4
AF = mybir.ActivationFunctionType
DR = mybir.MatmulPerfMode.DoubleRow


def build_bass(reps: int = 1):
    # reps>1 repeats the (idempotent) compute for device-time slope timing
    nc = _BaccOneActSet("TRN2", target_bir_lowering=False, debug=False)
    # All inputs pre-arranged on host into SBUF tile order: partition-major,
    # contiguous free dim -> DMA moves large contiguous runs per partition.
    # x1 is it-major so the first sim tile only needs a 128KB chunk.
    x1t = nc.dram_tensor("x1t", [P, IT * KT * P], FP8, kind="ExternalInput")
    z2t = nc.dram_tensor("z2t", [P, KT * N], FP8, kind="ExternalInput")
    a1t = nc.dram_tensor("a1t", [P, IT], F32, kind="ExternalInput")
    sdt = nc.dram_tensor("sdt", [P, IT], F32, kind="ExternalInput")
    out = nc.dram_tensor("row_loss", [P, IT], F32, kind="ExternalOutput")

    with tile.TileContext(nc) as tc:
        with (
            tc.tile_pool(name="x1", bufs=1) as x1pool,
            tc.tile_pool(name="z2", bufs=3) as z2pool,
            tc.tile_pool(name="dump", bufs=3) as dumppool,
            tc.tile_pool(name="small", bufs=1) as small,
            tc.tile_pool(name="psim", bufs=2, space="PSUM") as psim,
        ):
            # x1 + per-row scalars issue from the Scalar DGE queue so their
            # descriptor generation overlaps the Sync queue's z2 block 0.
            # The tiny a1/sdiag transfers go FIRST: issued later they'd sit
            # behind megabytes of z2 in the hardware queues and stall the
            # first exp (observed +11us).
            a1 = small.tile([P, IT], F32)
            nc.scalar.dma_start(out=a1, in_=a1t.ap())
            sdiag = small.tile([P, IT], F32)
            nc.scalar.dma_start(out=sdiag, in_=sdt.ap())
            x1s = x1pool.tile([P, IT, KT, P], FP8)
            x1r = x1t.ap().rearrange("p (h f) -> p h f", h=2)
            nc.scalar.dma_start(
                out=x1s[:, 0:IT // 2].rearrange("p a b c -> p (a b c)"),
                in_=x1r[:, 0, :],
            )
            nc.scalar.dma_start(
                out=x1s[:, IT // 2:].rearrange("p a b c -> p (a b c)"),
                in_=x1r[:, 1, :],
            )

            expsums = small.tile([P, IT, NBLK], F32)

            # ---- stream z2 by column blocks
            offs = np.cumsum([0] + BLKS)[:-1]
            sched = [(int(o), w) for o, w in zip(offs, BLKS)] * reps
            for jb, (off, w) in enumerate(sched):
                z2s = z2pool.tile([P, KT, w], FP8, tag=f"z2w{w}")
                nc.sync.dma_start(
                    out=z2s,
                    in_=z2t.ap()[:, KT * off:KT * (off + w)].rearrange(
                        "p (kt w) -> p kt w", kt=KT
                    ),
                )

                # ---- similarity block + exp + row-sum (fp8 DoubleRow)
                for it in range(IT):
                    sim = psim.tile([P, w], F32, tag="sim")
                    for ktp in range(KTP):
                        for h in range(w // MMW):
                            nc.tensor.matmul(
                                sim[:, h * MMW:(h + 1) * MMW],
                                x1s[:, it, 2 * ktp:2 * ktp + 2, :],
                                z2s[:, 2 * ktp:2 * ktp + 2,
                                    h * MMW:(h + 1) * MMW],
                                start=(ktp == 0),
                                stop=(ktp == KTP - 1),
                                perf_mode=DR,
                            )
                    dump = dumppool.tile([P, w], BF16, tag=f"dw{w}")
                    nc.scalar.activation(
                        dump, sim, AF.Exp, scale=a1[:, it:it + 1],
                    )
                    nc.vector.reduce_sum(
                        expsums[:, it, jb % NBLK:jb % NBLK + 1], dump,
                        axis=mybir.AxisListType.X,
                    )

            # ---- epilogue: row_loss = ln(sum_j exp) - s_diag
            s = small.tile([P, IT], F32)
            nc.vector.reduce_sum(s, expsums, axis=mybir.AxisListType.X)
            lse = small.tile([P, IT], F32)
            nc.scalar.activation(lse, s, AF.Ln)
            rl = small.tile([P, IT], F32)
            nc.vector.tensor_sub(rl, lse, sdiag)
            nc.sync.dma_start(out=out.ap(), in_=rl)

    nc.compile()
    return nc


_NC_CACHE = None
_LAST_RESULTS = None
_NORM_JIT = None


def _host_prep(view1: np.ndarray, view2: np.ndarray):
    """Normalize z2 once on host (the all-gather stand-in), quantize both
    operands to fp8, pre-arrange into SBUF tile order, and compute the
    per-row exp scales + similarity diagonal of the exact fp8 values."""
    global _NORM_JIT
    import jax
    import ml_dtypes

    fp8 = np.dtype(ml_dtypes.float8_e4m3)
    cpu = jax.devices("cpu")[0]
    if _NORM_JIT is None:
        import jax.numpy as jnp

        def _norm_t(v2):
            # [N, D] -> normalized, scaled, transposed [D, N]
            n = jnp.sqrt(jnp.sum(v2 * v2, axis=1, keepdims=True))
            z = v2 * (GAM / jnp.maximum(n, 1e-12))
            return z.T

        _NORM_JIT = jax.jit(_norm_t, backend="cpu")
    with jax.default_device(cpu):
        z2T = np.asarray(_NORM_JIT(view2))       # [D, N] f32
    z2T8 = z2T.astype(fp8)
    x1T8 = np.ascontiguousarray(
        np.asarray(view1, np.float32).T
    ).astype(fp8)                                # [D, N]

    x1f = x1T8.astype(np.float32)                # exact fp8 values
    z2f = z2T8.astype(np.float32)
    nsq1 = np.einsum("di,di->i", x1f, x1f)       # ||x1q_i||^2
    a1 = 1.0 / (8.0 * np.sqrt(nsq1))             # exp scale per row
    sdiag = a1 * np.einsum("di,di->i", x1f, z2f)  # true logit diagonal

    # z2: [D, N] -> block-major [P, sum_b KT*w_b]: each column block is one
    # contiguous kt-major run per partition (up to 16KB descriptors)
    offs = np.cumsum([0] + BLKS)[:-1]
    z2full = np.concatenate(
        [
            np.ascontiguousarray(
                z2T8[:, o:o + w].reshape(KT, P, w).transpose(1, 0, 2)
            ).reshape(P, KT * w)
            for o, w in zip(offs, BLKS)
        ],
        axis=1,
    )

    def x1_tiles(a):  # [D, NLOC] -> [P, IT, KT, 128] it-major
        return np.ascontiguousarray(
            a.reshape(KT, P, IT, P).transpose(1, 2, 0, 3)
        ).reshape(P, -1)

    def pcol(v):  # [NLOC] -> [P, IT] with row it*128+p at [p, it]
        return np.ascontiguousarray(v.reshape(IT, P).T.astype(np.float32))

    return x1T8, z2full, a1, sdiag, x1_tiles, pcol


def kernel(view1: np.ndarray, view2: np.ndarray) -> np.ndarray:
    global _NC_CACHE
    x1 = np.asarray(view1, dtype=np.float32)
    x2 = np.asarray(view2, dtype=np.float32)
    assert x1.shape == (N, D) and x2.shape == (N, D)

    x1T8, z2full, a1, sdiag, x1_tiles, pcol = _host_prep(x1, x2)

    in_maps = []
    for k in range(NC):
        sl = slice(k * NLOC, (k + 1) * NLOC)
        in_maps.append({
            "x1t": x1_tiles(x1T8[:, sl]),
            "z2t": z2full,
            "a1t": pcol(a1[sl]),
            "sdt": pcol(sdiag[sl]),
        })

    if _NC_CACHE is None:
        _NC_CACHE = build_bass()
    res = run_bass_kernel_spmd(_NC_CACHE, in_maps, core_ids=list(range(NC)))
    global _LAST_RESULTS
    _LAST_RESULTS = res

    total = 0.0
    for k in range(NC):
        total += res.results[k]["row_loss"].astype(np.float64).sum()
    return np.float32(total / N)


# revision 14
# speedup vs baseline: 1.0740x; 1.0633x over previous
"""Contrastive (InfoNCE) loss kernel for Trainium2, 8 NeuronCores.

Strategy (data-parallel over z1 rows, per the sharding hint):
  - Core k owns rows [k*1024, (k+1)*1024) of view1 and receives ALL of z2
    (view2's normalized form) -- one shared fp8 array, no per-core copy.
    z2 is normalized once on the host (the stand-in for "all-gather z2 or
    its normalized form"), scaled by 16, quantized to fp8e4m3, and
    pre-arranged into SBUF tile order so every DMA moves contiguous runs.
  - view1's slab ships RAW fp8 (quantize-then-normalize: the row norm of
    the quantized slab folds into the exp as the per-partition scale
    a1_i = 1/(8*||x1q_i||), so x1 needs NO on-device prescale pass).
    a1 and the similarity diagonal (both O(N*D) scalars of the exact fp8
    operands) are computed host-side and shipped as two [P, IT] tensors.
  - Per core the NEFF is a pure compute stream over column blocks
    (widths 512..2048 -- narrow at the edges to shorten the DMA ramp-in
    and the exp/reduce tail):
      sim tile [128, W] = x1_tile.T @ z2_tile  (fp8 DoubleRow matmuls,
        two 128-deep k-tiles per instruction, fp32 PSUM accum;
        sim = 16*||x1q_i||*cos)
      exp in one ACT op with per-partition scale AP a1[:, it]
        (no max subtraction: |logits| <= ~2.2); row-sum on DVE.
      row_loss = ln(sum_j exp) - s_diag
  - Host sums the 8192 per-row losses and divides by N.
"""

import numpy as np

import concourse.bass as bass
import concourse.mybir as mybir
import concourse.tile as tile
from concourse import bacc
from concourse.bass_utils import run_bass_kernel_spmd
from concourse.hw_specs import get_activation_tables


class _BaccOneActSet(bacc.Bacc):
    """Bacc whose act-table pass may only pick natural_log_exp_and_others.

    The default greedy picker ping-pongs between table sets, costing a
    ~1.3us table load per switch. Both functions used here (Exp, Ln) live
    in natural_log_exp_and_others, so masking the other sets (indices
    preserved) yields a single hoisted load.
    """

    ACT_SET = "natural_log_exp_and_others"

    def insert_act_table_loads(self):
        has_activation = any(
            isinstance(i, mybir.InstActivation)
            for b in self.main_func.blocks
            for i in b.instructions
        )
        if not has_activation:
            return
        tables = [
            (n, (s if n == self.ACT_SET else set()))
            for n, s in get_activation_tables(self.m.arch).items()
        ]
        bacc._bass_rust.insert_act_table_loads(self, tables)

N, D = 8192, 1024
NC = 8
NLOC = N // NC            # rows of view1 per core
P = 128                   # SBUF partitions
KT = D // P               # contraction tiles (128-deep)
KTP = KT // 2             # DoubleRow pairs of contraction tiles
IT = NLOC // P            # output row tiles per core
MMW = 512                 # PSUM free width per DoubleRow matmul
# similarity-column block widths; sum = N. Narrow head (start computing
# after a small DMA) and narrow tail (short exp/reduce epilogue).
BLKS = [512, 1536, 2048, 2048, 1536, 512]
assert sum(BLKS) == N
NBLK = len(BLKS)
GAM = 16.0                # fp8 headroom scale on z2
# sim PSUM value G = x1q . (GAM * z2hat) = GAM * ||x1q_i|| * cos
# logit = 2*cos = G / (8 * ||x1q_i||)  ->  exp scale a1_i = 1/(8*||x1q_i||)

F32 = mybir.dt.float32
BF16 = mybir.dt.bfloat16
FP8 = mybir.dt.float8e4
AF = mybir.ActivationFunctionType
DR = mybir.MatmulPerfMode.DoubleRow


def build_bass(reps: int = 1):
    # reps>1 repeats the (idempotent) compute for device-time slope timing
    nc = _BaccOneActSet("TRN2", target_bir_lowering=False, debug=False)
    # All inputs pre-arranged on host into SBUF tile order: partition-major,
    # contiguous free dim -> DMA moves large contiguous runs per partition.
    # x1 is it-major so the first sim tile only needs a 128KB chunk.
    x1t = nc.dram_tensor("x1t", [P, IT * KT * P], FP8, kind="ExternalInput")
    z2t = nc.dram_tensor("z2t", [P, KT * N], FP8, kind="ExternalInput")
    a1t = nc.dram_tensor("a1t", [P, IT], F32, kind="ExternalInput")
    sdt = nc.dram_tensor("sdt", [P, IT], F32, kind="ExternalInput")
    out = nc.dram_tensor("row_loss", [P, IT], F32, kind="ExternalOutput")

    with tile.TileContext(nc) as tc:
        with (
            tc.tile_pool(name="x1", bufs=1) as x1pool,
            tc.tile_pool(name="z2", bufs=3) as z2pool,
            tc.tile_pool(name="dump", bufs=3) as dumppool,
            tc.tile_pool(name="small", bufs=1) as small,
            tc.tile_pool(name="psim", bufs=2, space="PSUM") as psim,
        ):
            # The DMA hardware queues drain roughly FIFO, so issue order is
            # the schedule: tiny a1/sdiag first (a late issue would strand
            # their completion semaphores behind megabytes of z2), then x1
            # and z2 block 0 interleaved so the first sim tile can start
            # ~11us in, then the remaining z2 blocks.
            a1 = small.tile([P, IT], F32)
            nc.sync.dma_start(out=a1, in_=a1t.ap())
            sdiag = small.tile([P, IT], F32)
            nc.sync.dma_start(out=sdiag, in_=sdt.ap())
            x1s = x1pool.tile([P, IT, KT, P], FP8)
            x1r = x1t.ap().rearrange("p (h f) -> p h f", h=2)
            nc.sync.dma_start(
                out=x1s[:, 0:IT // 2].rearrange("p a b c -> p (a b c)"),
                in_=x1r[:, 0, :],
            )

            expsums = small.tile([P, IT, NBLK], F32)

            # ---- stream z2 by column blocks
            offs = np.cumsum([0] + BLKS)[:-1]
            sched = [(int(o), w) for o, w in zip(offs, BLKS)] * reps
            for jb, (off, w) in enumerate(sched):
                z2s = z2pool.tile([P, KT, w], FP8, tag=f"z2w{w}")
                nc.sync.dma_start(
                    out=z2s,
                    in_=z2t.ap()[:, KT * off:KT * (off + w)].rearrange(
                        "p (kt w) -> p kt w", kt=KT
                    ),
                )
                if jb == 0:
                    # second half of x1 rides between z2 blocks 0 and 1
                    nc.sync.dma_start(
                        out=x1s[:, IT // 2:].rearrange(
                            "p a b c -> p (a b c)"
                        ),
                        in_=x1r[:, 1, :],
                    )

                # ---- similarity block + exp + row-sum (fp8 DoubleRow)
                for it in range(IT):
                    sim = psim.tile([P, w], F32, tag="sim")
                    for ktp in range(KTP):
                        for h in range(w // MMW):
                            nc.tensor.matmul(
                                sim[:, h * MMW:(h + 1) * MMW],
                                x1s[:, it, 2 * ktp:2 * ktp + 2, :],
                                z2s[:, 2 * ktp:2 * ktp + 2,
                                    h * MMW:(h + 1) * MMW],
                                start=(ktp == 0),
                                stop=(ktp == KTP - 1),
                                perf_mode=DR,
                            )
                    dump = dumppool.tile([P, w], BF16, tag=f"dw{w}")
                    nc.scalar.activation(
                        dump, sim, AF.Exp, scale=a1[:, it:it + 1],
                    )
                    nc.vector.reduce_sum(
                        expsums[:, it, jb % NBLK:jb % NBLK + 1], dump,
                        axis=mybir.AxisListType.X,
                    )

            # ---- epilogue: row_loss = ln(sum_j exp) - s_diag
            s = small.tile([P, IT], F32)
            nc.vector.reduce_sum(s, expsums, axis=mybir.AxisListType.X)
            lse = small.tile([P, IT], F32)
            nc.scalar.activation(lse, s, AF.Ln)
            rl = small.tile([P, IT], F32)
            nc.vector.tensor_sub(rl, lse, sdiag)
            nc.sync.dma_start(out=out.ap(), in_=rl)

    nc.compile()
    return nc


_NC_CACHE = None
_LAST_RESULTS = None
_NORM_JIT = None


def _host_prep(view1: np.ndarray, view2: np.ndarray):
    """Normalize z2 once on host (the all-gather stand-in), quantize both
    operands to fp8, pre-arrange into SBUF tile order, and compute the
    per-row exp scales + similarity diagonal of the exact fp8 values."""
    global _NORM_JIT
    import jax
    import ml_dtypes

    fp8 = np.dtype(ml_dtypes.float8_e4m3)
    cpu = jax.devices("cpu")[0]
    if _NORM_JIT is None:
        import jax.numpy as jnp

        def _norm_t(v2):
            # [N, D] -> normalized, scaled, transposed [D, N]
            n = jnp.sqrt(jnp.sum(v2 * v2, axis=1, keepdims=True))
            z = v2 * (GAM / jnp.maximum(n, 1e-12))
            return z.T

        _NORM_JIT = jax.jit(_norm_t, backend="cpu")
    with jax.default_device(cpu):
        z2T = np.asarray(_NORM_JIT(view2))       # [D, N] f32
    z2T8 = z2T.astype(fp8)
    x1T8 = np.ascontiguousarray(
        np.asarray(view1, np.float32).T
    ).astype(fp8)                                # [D, N]

    x1f = x1T8.astype(np.float32)                # exact fp8 values
    z2f = z2T8.astype(np.float32)
    nsq1 = np.einsum("di,di->i", x1f, x1f)       # ||x1q_i||^2
    a1 = 1.0 / (8.0 * np.sqrt(nsq1))             # exp scale per row
    sdiag = a1 * np.einsum("di,di->i", x1f, z2f)  # true logit diagonal

    # z2: [D, N] -> block-major [P, sum_b KT*w_b]: each column block is one
    # contiguous kt-major run per partition (up to 16KB descriptors)
    offs = np.cumsum([0] + BLKS)[:-1]
    z2full = np.concatenate(
        [
            np.ascontiguousarray(
                z2T8[:, o:o + w].reshape(KT, P, w).transpose(1, 0, 2)
            ).reshape(P, KT * w)
            for o, w in zip(offs, BLKS)
        ],
        axis=1,
    )

    def x1_tiles(a):  # [D, NLOC] -> [P, IT, KT, 128] it-major
        return np.ascontiguousarray(
            a.reshape(KT, P, IT, P).transpose(1, 2, 0, 3)
        ).reshape(P, -1)

    def pcol(v):  # [NLOC] -> [P, IT] with row it*128+p at [p, it]
        return np.ascontiguousarray(v.reshape(IT, P).T.astype(np.float32))

    return x1T8, z2full, a1, sdiag, x1_tiles, pcol


def kernel(view1: np.ndarray, view2: np.ndarray) -> np.ndarray:
    global _NC_CACHE
    x1 = np.asarray(view1, dtype=np.float32)
    x2 = np.asarray(view2, dtype=np.float32)
    assert x1.shape == (N, D) and x2.shape == (N, D)

    x1T8, z2full, a1, sdiag, x1_tiles, pcol = _host_prep(x1, x2)

    in_maps = []
    for k in range(NC):
        sl = slice(k * NLOC, (k + 1) * NLOC)
        in_maps.append({
            "x1t": x1_tiles(x1T8[:, sl]),
            "z2t": z2full,
            "a1t": pcol(a1[sl]),
            "sdt": pcol(sdiag[sl]),
        })

    if _NC_CACHE is None:
        _NC_CACHE = build_bass()
    res = run_bass_kernel_spmd(_NC_CACHE, in_maps, core_ids=list(range(NC)))
    global _LAST_RESULTS
    _LAST_RESULTS = res

    total = 0.0
    for k in range(NC):
        total += res.results[k]["row_loss"].astype(np.float64).sum()
    return np.float32(total / N)
